# revision 1
# baseline (speedup 1.0000x reference)
"""AttentivePooling Trainium2 kernel (8 NeuronCores, SPMD).

Math (per graph g):  pooled[g] = sum_{n in g} softmax_g(s)_n * x[n]
with s_n = tanh(x W1 + b1) W2 + b2.  Since tanh bounds |s| <= ||W2||_1 + |b2|
(~9 for these inputs), the segment-max subtraction in the reference is
unnecessary: we accumulate  num[g] = sum exp(s_n - SHIFT) x_n  and
den[g] = sum exp(s_n - SHIFT)  in one streaming pass and divide at the end
(the SHIFT cancels).

Sharding: 2048 graphs -> 8 cores x 8 groups x 32 graphs. Node rows of each
group are host-packed contiguously and padded to a common capacity C so all
cores run one identical NEFF. Per 128-node tile the device:
  - computes h^T = tanh(W1^T x^T + b1) from a host-prepared transposed copy
    of x (PE matmul, contraction over hidden dim needs hid on partitions),
  - scores s = h^T.T @ W2 as a [128,1] column, ex = exp(s + b2 - SHIFT),
  - builds A[n, j] = ex_n * (iota_j == batch_rel_n) with one fused DVE op,
  - accumulates pooled^groupT += A.T @ x_aug into PSUM, where x_aug has a
    ones column appended so column 256 accumulates the denominator.
"""

import os
import sys

for _p in ("/opt/trn_rl_repo",):
    if _p not in sys.path:
        sys.path.insert(0, _p)

import numpy as np

# ---------------------------------------------------------------- geometry
N_NODES = 1048576
HID = 256
HID2 = 128
G_TOTAL = 2048
N_CORES = 8
GT = 32            # graphs per pooling group (PSUM partition dim of pooled)
NG = 8             # groups per core
SEGS_PER_CORE = NG * GT          # 256
XW = HID + 4       # x_aug row width: 256 features + 1.0 + 3 pad zeros
ONES_COL = HID     # column index of the ones column
CHUNK_TILES = 16   # 128-node tiles per DMA chunk
ST = 4             # tiles per score supertile (512 nodes)

# ---------------------------------------------------------------- dtypes
USE_FP16 = os.environ.get("KERNEL_FP16", "1") == "1"
SHIFT = 8.0 if USE_FP16 else 0.0

_nc_cache = {}


def _dts():
    import concourse.mybir as mybir
    return mybir.dt.float16 if USE_FP16 else mybir.dt.float32


def _np_dts():
    return np.float16 if USE_FP16 else np.float32


# ================================================================ device IR
def build_bass(ntpg, ngroups=NG, gt=GT, use_fp16=None):
    """Build + compile the per-core Bass program.

    ntpg: 128-node tiles per group (group capacity C = ntpg*128), mult of 4.
    """
    import concourse.bacc as bacc
    import concourse.mybir as mybir
    import concourse.tile as tile

    if use_fp16 is None:
        use_fp16 = USE_FP16
    dts = mybir.dt.float16 if use_fp16 else mybir.dt.float32
    f32 = mybir.dt.float32
    AF = mybir.ActivationFunctionType
    OP = mybir.AluOpType

    assert ntpg % ST == 0
    T = ngroups * ntpg                  # tiles per core
    S = T * 128                         # padded nodes per core

    nc = bacc.Bacc("TRN2", num_devices=N_CORES)

    xa = nc.dram_tensor("xa", [S, XW], dts, kind="ExternalInput").ap()
    xt = nc.dram_tensor("xt", [HID, S], dts, kind="ExternalInput").ap()
    crel = nc.dram_tensor("crel", [128, T], f32, kind="ExternalInput").ap()
    w1 = nc.dram_tensor("w1", [HID, HID2], dts, kind="ExternalInput").ap()
    w2 = nc.dram_tensor("w2", [HID2, 1], dts, kind="ExternalInput").ap()
    b1c = nc.dram_tensor("b1c", [HID2, 1], f32, kind="ExternalInput").ap()
    b2c = nc.dram_tensor("b2c", [128, 1], f32, kind="ExternalInput").ap()
    iota = nc.dram_tensor("iota", [128, gt], dts, kind="ExternalInput").ap()
    out = nc.dram_tensor("out", [ngroups * gt, HID], f32, kind="ExternalOutput").ap()

    with tile.TileContext(nc) as tc:
        with (
            tc.tile_pool(name="consts", bufs=1) as cpool,
            tc.tile_pool(name="xa", bufs=3) as xa_pool,
            tc.tile_pool(name="xt", bufs=3) as xt_pool,
            tc.tile_pool(name="th", bufs=3) as th_pool,
            tc.tile_pool(name="ex", bufs=3) as ex_pool,
            tc.tile_pool(name="amat", bufs=4) as a_pool,
            tc.tile_pool(name="fin", bufs=2) as fin_pool,
            tc.tile_pool(name="hp", bufs=2, space="PSUM") as hp_pool,
            tc.tile_pool(name="sp", bufs=2, space="PSUM") as sp_pool,
            tc.tile_pool(name="pp", bufs=2, space="PSUM") as pp_pool,
        ):
            # ---- resident constants
            w1_sb = cpool.tile([128, 2 * HID2], dts)
            nc.sync.dma_start(out=w1_sb[:, 0:HID2], in_=w1[0:128, :])
            nc.sync.dma_start(out=w1_sb[:, HID2:2 * HID2], in_=w1[128:256, :])
            w2_sb = cpool.tile([128, 1], dts)
            nc.sync.dma_start(out=w2_sb[:], in_=w2[:])
            b1_sb = cpool.tile([128, 1], f32)
            nc.sync.dma_start(out=b1_sb[:], in_=b1c[:])
            b2_sb = cpool.tile([128, 1], f32)
            nc.sync.dma_start(out=b2_sb[:], in_=b2c[:])
            iota_sb = cpool.tile([128, gt], dts)
            nc.sync.dma_start(out=iota_sb[:], in_=iota[:])
            crel_sb = cpool.tile([128, T], f32)
            nc.sync.dma_start(out=crel_sb[:], in_=crel[:])

            for g in range(ngroups):
                pool_ps = pp_pool.tile([gt, XW], f32, space="PSUM", tag="pool")
                for c0 in range(0, ntpg, CHUNK_TILES):
                    nt = min(CHUNK_TILES, ntpg - c0)
                    node0 = (g * ntpg + c0) * 128
                    xa_sb = xa_pool.tile([128, nt * XW], dts, tag="xa")
                    nc.sync.dma_start(
                        out=xa_sb[:].rearrange("p (t d) -> p t d", d=XW),
                        in_=xa[node0:node0 + nt * 128, :].rearrange(
                            "(t p) d -> p t d", p=128),
                    )
                    xt0_sb = xt_pool.tile([128, nt * 128], dts, tag="xt0")
                    nc.sync.dma_start(
                        out=xt0_sb[:], in_=xt[0:128, node0:node0 + nt * 128])
                    xt1_sb = xt_pool.tile([128, nt * 128], dts, tag="xt1")
                    nc.sync.dma_start(
                        out=xt1_sb[:], in_=xt[128:256, node0:node0 + nt * 128])

                    for st in range(nt // ST):
                        w = ST * 128  # 512 nodes
                        hp = hp_pool.tile([128, w], f32, space="PSUM", tag="hp")
                        nc.tensor.matmul(
                            out=hp[:], lhsT=w1_sb[:, 0:HID2],
                            rhs=xt0_sb[:, st * w:(st + 1) * w],
                            start=True, stop=False)
                        nc.tensor.matmul(
                            out=hp[:], lhsT=w1_sb[:, HID2:2 * HID2],
                            rhs=xt1_sb[:, st * w:(st + 1) * w],
                            start=False, stop=True)
                        th = th_pool.tile([128, w], dts, tag="th")
                        nc.scalar.activation(th[:], hp[:], AF.Tanh,
                                             bias=b1_sb[:, 0:1])
                        sp = sp_pool.tile([128, ST], f32, space="PSUM", tag="sp")
                        for j in range(ST):
                            nc.tensor.matmul(
                                out=sp[:, j:j + 1],
                                lhsT=th[:, j * 128:(j + 1) * 128],
                                rhs=w2_sb[:],
                                start=(j == 0), stop=(j == ST - 1),
                                skip_group_check=True)
                        ex = ex_pool.tile([128, ST], f32, tag="ex")
                        nc.scalar.activation(ex[:], sp[:], AF.Exp,
                                             bias=b2_sb[:, 0:1])
                        for j in range(ST):
                            t_in_g = c0 + st * ST + j
                            t_abs = g * ntpg + t_in_g
                            amat = a_pool.tile([128, gt], dts, tag="amat")
                            nc.vector.tensor_scalar(
                                amat[:], iota_sb[:],
                                crel_sb[:, t_abs:t_abs + 1],
                                ex[:, j:j + 1],
                                OP.is_equal, OP.mult)
                            nc.tensor.matmul(
                                out=pool_ps[:],
                                lhsT=amat[:],
                                rhs=xa_sb[:, t_in_g * XW - c0 * XW:
                                          (t_in_g - c0 + 1) * XW],
                                start=(t_in_g == 0), stop=(t_in_g == ntpg - 1),
                                skip_group_check=True)

                # ---- finalize group: pooled = num / den
                rec = fin_pool.tile([gt, 1], f32, tag="rec")
                nc.vector.reciprocal(rec[:], pool_ps[:, ONES_COL:ONES_COL + 1])
                og = fin_pool.tile([gt, HID], f32, tag="og")
                nc.vector.tensor_scalar(
                    og[:], pool_ps[:, 0:HID], rec[:, 0:1], None, OP.mult)
                nc.sync.dma_start(out=out[g * gt:(g + 1) * gt, :], in_=og[:])

    nc.compile()
    return nc


# ================================================================ host prep
def prepare_shards(x, batch, W1, b1, W2, b2, ngroups=NG, gt=GT, n_cores=N_CORES):
    """Split nodes into (core, group) node blocks padded to capacity C."""
    np_dts = _np_dts()
    x = np.asarray(x)
    batch = np.asarray(batch).astype(np.int64)
    g_total = n_cores * ngroups * gt
    counts = np.bincount(batch, minlength=g_total)
    n_groups_total = n_cores * ngroups
    gcounts = counts.reshape(n_groups_total, gt).sum(1)
    C = int(max(512, ((int(gcounts.max()) + ST * 128 - 1) // (ST * 128)) * ST * 128))
    ntpg = C // 128
    T = ngroups * ntpg
    gstart = np.concatenate([[0], np.cumsum(gcounts)])[:-1]

    w1c = np.ascontiguousarray(W1).astype(np_dts)
    w2c = np.ascontiguousarray(W2).astype(np_dts)
    b1c = np.asarray(b1, np.float32).reshape(HID2, 1)
    b2c = np.full((128, 1), float(np.asarray(b2).reshape(-1)[0]) - SHIFT,
                  np.float32)
    iota = np.tile(np.arange(gt, dtype=np.float32), (128, 1)).astype(np_dts)

    in_maps = []
    for core in range(n_cores):
        xa = np.zeros((ngroups * C, XW), np.float32)
        crel_flat = np.full(ngroups * C, -1.0, np.float32)
        for g in range(ngroups):
            gid = core * ngroups + g
            s0, n = int(gstart[gid]), int(gcounts[gid])
            xa[g * C:g * C + n, :HID] = x[s0:s0 + n]
            crel_flat[g * C:g * C + n] = (
                batch[s0:s0 + n] - (core * ngroups + g) * gt).astype(np.float32)
        xa[:, ONES_COL] = 1.0
        xt = np.ascontiguousarray(xa[:, :HID].T).astype(np_dts)
        in_maps.append({
            "xa": xa.astype(np_dts),
            "xt": xt,
            "crel": np.ascontiguousarray(crel_flat.reshape(T, 128).T),
            "w1": w1c, "w2": w2c, "b1c": b1c, "b2c": b2c, "iota": iota,
        })
    return in_maps, ntpg


# ================================================================ entry
LAST_RESULTS = None


def kernel(x, batch, W1, b1, W2, b2):
    global LAST_RESULTS
    from concourse.bass_utils import run_bass_kernel_spmd

    in_maps, ntpg = prepare_shards(x, batch, W1, b1, W2, b2)
    key = (ntpg, USE_FP16)
    if key not in _nc_cache:
        _nc_cache[key] = build_bass(ntpg)
    nc = _nc_cache[key]
    trace = os.environ.get("KERNEL_TRACE", "0") == "1"
    res = run_bass_kernel_spmd(nc, in_maps, core_ids=list(range(N_CORES)),
                               trace=trace)
    LAST_RESULTS = res
    pooled = np.concatenate([r["out"] for r in res.results], axis=0)
    return pooled.astype(np.float32)
